# revision 1
# baseline (speedup 1.0000x reference)
"""Multi-head attention (B=2, S=2048, H=1024, NH=16) on 8 TRN2 NeuronCores.

Sharding: core c -> (batch b = c//4, head-group hg = c%4). Each core computes
Q/K/V projections for its 4 heads (256 columns of Wq/Wk/Wv), attention for
those heads, and a partial output projection (its 256 rows of Wo, bias bo/4).
Host sums the 4 partials per batch.

Per-core device pipeline (all matmuls at 1 cycle/row via float32r/bf16):
  - x is pre-transposed + bf16-cast on the host, so h-major xT streams
    straight into the projections.
  - Q/K projections produce qT/kT d-major [256, 2048] f32r (W stationary);
    V s-major [2048, 4, 65] bf16 (xT stationary) with a ones column appended.
  - scoresT[sk, sq] per head: lhsT = kT head slice (K=64; head pairs sit at
    base partitions 0/64).
  - exp on ACT reads score PSUM directly (scale=1/8 fused), writes bf16,
    both heads per instruction (FD=1024 amortizes the ACT per-op overhead).
  - AV in [sq, d] orientation: lhsT = et 128-col chunk (full K=128, M=128),
    rhs = v+ones [128, 65] -> psum [sq 128, 65], 65-cycle matmuls; col 64
    accumulates the softmax denominators for free.
  - Normalization: denominators land per-partition, so a batched DVE
    reciprocal + one TensorScalarPtr multiply per (head, sq-chunk) writes
    normalized attn [sq, d] bf16 straight out of PSUM.
  - attn -> attnT via DMA XBAR transpose (idle DMA engines; no PE/PSUM).
  - Output projection all-bf16: attnT stationary, Wo rows moving.
PSUM budget (8 banks): 2 projection/misc + 2x2 score double-buffer + 2 AV.
"""
import os
import sys

if os.path.isdir("/opt/trn_rl_repo"):
    sys.path.insert(0, "/opt/trn_rl_repo")

from contextlib import ExitStack

import numpy as np
import ml_dtypes

import concourse.bass as bass
import concourse.tile as tile
from concourse import bacc, mybir
from concourse.bass import ts
from concourse.bass_utils import run_bass_kernel_spmd

F32 = mybir.dt.float32
F32R = mybir.dt.float32r
BF16 = mybir.dt.bfloat16
EXP = mybir.ActivationFunctionType.Exp

S = 2048
H = 1024
D = 256          # per-core head-slice width (4 heads x 64)
HD = 64
N_CORES = 8
SB = 512         # s-block
NSB = S // SB    # 4
HT = H // 128    # 8 h-tiles
SKT = S // 128   # 16 sk-tiles
SCALE = 1.0 / 8.0  # 1/sqrt(HD)

_CACHE = {}


def _build():
    nc = bacc.Bacc("TRN2", target_bir_lowering=False, debug=False,
                   num_devices=N_CORES)

    xq = nc.dram_tensor("xqT", [H, S], BF16, kind="ExternalInput").ap()
    xk = nc.dram_tensor("xkT", [H, S], BF16, kind="ExternalInput").ap()
    xv = nc.dram_tensor("xvT", [H, S], BF16, kind="ExternalInput").ap()
    wq_d = nc.dram_tensor("wq", [H, D], BF16, kind="ExternalInput").ap()
    wk_d = nc.dram_tensor("wk", [H, D], BF16, kind="ExternalInput").ap()
    wv_d = nc.dram_tensor("wv", [H, D], BF16, kind="ExternalInput").ap()
    wo_d = nc.dram_tensor("wo", [D, H], BF16, kind="ExternalInput").ap()
    bq_d = nc.dram_tensor("bq2", [128, 2], F32, kind="ExternalInput").ap()
    bk_d = nc.dram_tensor("bk2", [128, 2], F32, kind="ExternalInput").ap()
    bv_d = nc.dram_tensor("bv1", [1, D], F32, kind="ExternalInput").ap()
    bo_d = nc.dram_tensor("bo4", [1, H], F32, kind="ExternalInput").ap()
    y = nc.dram_tensor("y", [S, H], F32, kind="ExternalOutput").ap()

    with tile.TileContext(nc) as tc:
        with ExitStack() as ctx:
            const = ctx.enter_context(tc.tile_pool(name="const", bufs=1))
            pers = ctx.enter_context(tc.tile_pool(name="pers", bufs=1))
            xt_p = ctx.enter_context(tc.tile_pool(name="xt", bufs=2))
            small = ctx.enter_context(tc.tile_pool(name="small", bufs=4))
            exp_p = ctx.enter_context(tc.tile_pool(name="expp", bufs=40))
            hold_p = ctx.enter_context(tc.tile_pool(name="holdp", bufs=16))
            atn_p = ctx.enter_context(tc.tile_pool(name="atnp", bufs=6))
            fin_p = ctx.enter_context(tc.tile_pool(name="finp", bufs=2))

            # ---- constants ----
            # only wq's DMA goes ahead of the critical xq0/xk0 loads; the
            # small bias transfers would hold the HWDGE ~0.6us each.
            wq = const.tile([128, HT, D], BF16)
            nc.sync.dma_start(wq[:], wq_d.rearrange("(j p) d -> p j d", p=128))
            wk = const.tile([128, HT, D], BF16)
            wv = const.tile([128, HT, D], BF16)
            bq2 = const.tile([128, 2], F32)
            bk2 = const.tile([128, 2], F32)
            bv1 = const.tile([1, D], F32)
            ones_f = const.tile([1, 128], F32)
            nc.gpsimd.memset(ones_f[:], 1.0)
            ones = const.tile([1, 128], F32R)
            nc.vector.tensor_copy(ones[:], ones_f[:])
            bv1r = const.tile([1, D], F32R)
            warm = const.tile([1, 2], BF16)
            nc.scalar.activation(warm[:], ones_f[0:1, 0:2], EXP)

            # ---- persistent activations ----
            qT = pers.tile([128, 2, S], BF16)   # [d_local, dh, s]
            kT = pers.tile([128, 2, S], BF16)
            vS = pers.tile([128, SKT, 4, HD + 1], BF16)  # [sk, sk_tile, head, d|1]
            nc.gpsimd.memset(vS[:], 1.0)       # ones column (rest overwritten)
            attnT = pers.tile([128, 2, S], BF16)  # [d in pair, hp, sq]

            ps_pj = ctx.enter_context(
                tc.tile_pool(name="ps_pj", bufs=2, space="PSUM"))
            ps_qk = ctx.enter_context(
                tc.tile_pool(name="ps_qk", bufs=2, space="PSUM"))
            ps_av = ctx.enter_context(
                tc.tile_pool(name="ps_av", bufs=2, space="PSUM"))

            bvb = const.tile([128, D], F32)

            def load_xt(xd, sb, name):
                """DMA one s-block of pre-transposed x: [128h, HT, SB] bf16."""
                xt = xt_p.tile([128, HT, SB], BF16, tag="xt", name=name)
                nc.sync.dma_start(
                    xt[:], xd.rearrange("(j p) s -> p j s", p=128)[
                        :, :, ts(sb, SB)])
                return xt

            def proj_dmajor_unit(xt, w, bias2, dst, sb, dh, c0=0, c1=SB):
                # dst[:, dh, sb*SB+c0:+c1] = (x @ w + b).T (d-major)
                pp = ps_pj.tile([128, 512], F32, tag="pj", name="pp")
                for j in range(HT):
                    nc.tensor.matmul(pp[:, 0:c1 - c0], w[:, j, ts(dh, 128)],
                                     xt[:, j, c0:c1],
                                     start=(j == 0), stop=(j == HT - 1))
                nc.vector.tensor_scalar_add(
                    dst[:, dh, sb * SB + c0:sb * SB + c1], pp[:, 0:c1 - c0],
                    bias2[:, dh:dh + 1])

            def qk_exp(hp, sqb, sk, pool=None):
                pqk = ps_qk.tile([128, 2, 512], F32, tag="qk", name="pqk")
                for hh in range(2):
                    r0 = HD * hh
                    nc.tensor.matmul(
                        pqk[:, hh, :],
                        kT[r0:r0 + HD, hp, ts(sk, 128)],
                        qT[r0:r0 + HD, hp, ts(sqb, SB)],
                        start=True, stop=True)
                et = (pool or exp_p).tile([128, 2, 512], BF16, tag="e",
                                          name="et")
                nc.scalar.activation(et[:], pqk[:], EXP, scale=SCALE)
                return et

            def av_accum(hp, sk, et, pav):
                # pav[hh][:, sqc, 0:65] += et[hh][:,chunk].T @ v+ones
                # start=True marks the whole 2KB PSUM bank pending-zero, so
                # only the bank's first group may use it; later groups'
                # first accumulate reads pending-zero bytes as zero.
                for hh in range(2):
                    for sqc in range(4):
                        nc.tensor.matmul(
                            pav[hh][:, sqc, 0:HD + 1],
                            et[:, hh, ts(sqc, 128)],
                            vS[:, sk, 2 * hp + hh, :],
                            start=(sk == 0 and sqc == 0),
                            stop=(sk == SKT - 1),
                            skip_group_check=True)

            def attn_group(hp, sqb, sk, pav):
                av_accum(hp, sk, qk_exp(hp, sqb, sk), pav)

            def emit_outproj_chunk(sqb, st, tail=False):
                # tail=True (the final sq block): ACT is out of exp work, so
                # fold the bias in as a ones-row accumulation on PE and move
                # PSUM->SBUF on the scalar engine, keeping DVE off the
                # drain-critical path.
                fin = fin_p.tile([128, H], F32, tag="fin", name="fin")
                for eb in range(2):
                    po = ps_pj.tile([128, 512], F32, tag="pj", name="po")
                    nc.tensor.matmul(po[:],
                                     attnT[:, 0, ts(4 * sqb + st, 128)],
                                     wo[:, 0, ts(eb, 512)],
                                     start=True, stop=False,
                                     skip_group_check=True)
                    nc.tensor.matmul(po[:],
                                     attnT[:, 1, ts(4 * sqb + st, 128)],
                                     wo[:, 1, ts(eb, 512)],
                                     start=False, stop=not tail,
                                     skip_group_check=True)
                    if tail:
                        nc.tensor.matmul(po[:], ones[0:1, :],
                                         bo4r[:, ts(eb, 512)],
                                         start=False, stop=True,
                                         skip_group_check=True)
                        nc.scalar.activation(
                            fin[:, ts(eb, 512)], po[:],
                            mybir.ActivationFunctionType.Copy)
                    else:
                        nc.vector.tensor_add(fin[:, ts(eb, 512)], po[:],
                                             bob[:, ts(eb, 512)])
                nc.sync.dma_start(y[ts(4 * sqb + st, 128), :], fin[:])

            def emit_outproj(sqb):
                for st in range(4):
                    emit_outproj_chunk(sqb, st)

            def attn_finish(hp, sqb, pav, fuse_outproj=False):
                # normalize out of PSUM (per-partition denominators in col 64)
                # and transpose [sq, d] -> attnT[d, sq] on the DMA XBAR.
                atn = [atn_p.tile([128, 128], BF16, tag="atn", name="atn")
                       for _ in range(4)]
                recs = []
                for hh in range(2):
                    rec = small.tile([128, 4], F32, tag="rec", name="rec")
                    with nc.allow_low_precision(reason="softmax denom recip"):
                        nc.vector.reciprocal(rec[:], pav[hh][:, :, HD:HD + 1])
                    recs.append(rec)
                for sqc in range(4):
                    for hh in range(2):
                        nc.vector.tensor_scalar_mul(
                            atn[sqc][:, ts(hh, HD)],
                            pav[hh][:, sqc, 0:HD],
                            recs[hh][:, sqc:sqc + 1])
                    # issue on ACT's queue: it reaches this right after the
                    # batch's last exp dispatch so the wait is short, and its
                    # 4-deep wait queue keeps the exp engine fed meanwhile.
                    # On SP the wait would head-of-line-block y-DMAs ~15us.
                    nc.scalar.dma_start_transpose(
                        attnT[:, hp, ts(4 * sqb + sqc, 128)], atn[sqc][:])
                    if fuse_outproj:
                        emit_outproj_chunk(sqb, sqc, tail=True)

            # ---- streaming loads + projections, with scores+exp for ready
            # (sqb, hp, sk) tiles pre-emitted in consumption order so ACT
            # starts chewing softmax exps ~10us in and never starves.
            # Per-batch precompute counts: front batches lean on the
            # rotating pool (consumed within ~2.5 batches, before buffer
            # reuse); later batches' tiles are held to the end of the
            # attention phase and must live in the non-rotating hold pool.
            def spread(p):
                # evenly-spaced sk positions so pre-AVs interleave with
                # inline groups inside a batch (ACT never idles in bursts)
                return sorted({int(round(i * SKT / p)) for i in range(p)})

            PRE_SPEC = [((0, 0), list(range(SKT)), None),
                        ((0, 1), spread(8), None),
                        ((1, 0), spread(8), None),
                        ((1, 1), spread(8), None),
                        ((2, 0), spread(8), hold_p),
                        ((2, 1), spread(6), hold_p),
                        ((3, 0), spread(2), hold_p)]
            pre_order = [(sqb, hp, sk, pool)
                         for (sqb, hp), sks, pool in PRE_SPEC
                         for sk in sks]
            pre = {}
            st_pre = {"i": 0, "q": set(), "k": set()}

            def emit_pre(limit):
                # dh-granular readiness: head-pair hp only needs the dh=hp
                # halves of its qT/kT blocks.
                done = 0
                while st_pre["i"] < len(pre_order) and done < limit:
                    sqb, hp, sk, pool = pre_order[st_pre["i"]]
                    if (sqb, hp) not in st_pre["q"] or \
                            (sk // 4, hp) not in st_pre["k"]:
                        break
                    pre[(sqb, hp, sk)] = qk_exp(hp, sqb, sk, pool)
                    st_pre["i"] += 1
                    done += 1

            def proj_qk(xd, w, bias2, dst, sb, which):
                xt = load_xt(xd, sb, "xt" + which)
                for dh in range(2):
                    proj_dmajor_unit(xt, w, bias2, dst, sb, dh)
                    st_pre[which].add((sb, dh))
                    emit_pre(3)

            # batch (0,0)'s AV interleaves into the V phase: its et tiles
            # are all precomputed, and vS[sk] is ready right after block
            # sk//4's epilogue -- so the first attention batch costs no
            # wall-clock of its own and inline exps resume at proj end.
            pav00 = [ps_av.tile([128, 4, 128], F32, tag="av", name=f"pav{hh}")
                     for hh in range(2)]

            def proj_v(sb):
                # batch (0,0)'s AV trails the V epilogues by one si unit so
                # the in-order PE never waits on the DVE vS write latency.
                xtv = load_xt(xv, sb, "xtv")
                for si in range(4):
                    pv = ps_pj.tile([128, 512], F32, tag="pj", name="pv")
                    for j in range(HT):
                        nc.tensor.matmul(pv[:, 0:D],
                                         xtv[:, j, ts(si, 128)],
                                         wv[:, j, :],
                                         start=(j == 0), stop=(j == HT - 1))
                    nc.vector.tensor_add(
                        vS[:, 4 * sb + si, :, 0:HD],
                        pv[:, 0:D].rearrange("p (g d) -> p g d", g=4),
                        bvb[:].rearrange("p (g d) -> p g d", g=4))
                    emit_pre(3)
                    sk = 4 * sb + si
                    if sk > 0:
                        av_accum(0, sk - 1, pre.pop((0, 0, sk - 1)), pav00)

            # Q0 then all K (unlocks every sqb0 tile), then Q1-3 (unlocks
            # the rest), V last (first consumed only once attention starts).
            xtq0 = load_xt(xq, 0, "xtq")
            nc.sync.dma_start(wk[:], wk_d.rearrange("(j p) d -> p j d", p=128))
            # fast start: narrow first xk load + mini K projection puts the
            # first score+exp on ACT ~8us in instead of ~16us.
            xtk0a = xt_p.tile([128, HT, 128], BF16, tag="xta", name="xtk0a")
            nc.sync.dma_start(
                xtk0a[:], xk.rearrange("(j p) s -> p j s", p=128)[:, :, 0:128])
            nc.sync.dma_start(bq2[:], bq_d[:])
            nc.sync.dma_start(bk2[:], bk_d[:])
            for dh in range(2):
                proj_dmajor_unit(xtq0, wq, bq2, qT, 0, dh)
            st_pre["q"].add((0, 0))
            st_pre["q"].add((0, 1))
            pk0 = ps_pj.tile([128, 512], F32, tag="pj", name="pk0")
            for j in range(HT):
                nc.tensor.matmul(pk0[:, 0:128], wk[:, j, 0:128],
                                 xtk0a[:, j, :],
                                 start=(j == 0), stop=(j == HT - 1))
            nc.vector.tensor_scalar_add(kT[:, 0, 0:128], pk0[:, 0:128],
                                        bk2[:, 0:1])
            pre[(0, 0, 0)] = qk_exp(0, 0, 0)
            st_pre["i"] = 1
            xtk0 = load_xt(xk, 0, "xtk")
            proj_dmajor_unit(xtk0, wk, bk2, kT, 0, 0, 128, SB)
            st_pre["k"].add((0, 0))
            emit_pre(3)
            proj_dmajor_unit(xtk0, wk, bk2, kT, 0, 1)
            st_pre["k"].add((0, 1))
            emit_pre(3)
            for sb in range(1, NSB):
                proj_qk(xk, wk, bk2, kT, sb, "k")
                emit_pre(3)
            for sb in range(1, NSB):
                proj_qk(xq, wq, bq2, qT, sb, "q")
                emit_pre(3)
            nc.sync.dma_start(wv[:], wv_d.rearrange("(j p) d -> p j d", p=128))
            # v-bias broadcast, deferred here so its small DMAs stay off the
            # critical startup path (first needed by V0's epilogue)
            nc.sync.dma_start(bv1[:], bv_d[:])
            nc.vector.tensor_copy(bv1r[:], bv1[:])
            pbc = ps_pj.tile([128, 512], F32, tag="pj", name="pbc")
            nc.tensor.matmul(pbc[:, 0:D], ones[0:1, :], bv1r[:])
            nc.vector.tensor_copy(bvb[:], pbc[:, 0:D])
            for sb in range(NSB):
                proj_v(sb)
            av_accum(0, SKT - 1, pre.pop((0, 0, SKT - 1)), pav00)
            emit_pre(len(pre_order))

            # deferred: output-projection constants (first needed ~60us in)
            wo = const.tile([128, 2, H], BF16)
            nc.sync.dma_start(wo[:], wo_d.rearrange("(i p) e -> p i e", p=128))
            bo4 = const.tile([1, H], F32)
            nc.sync.dma_start(bo4[:], bo_d[:])
            bo4r = const.tile([1, H], F32R)
            nc.vector.tensor_copy(bo4r[:], bo4[:])
            bob = const.tile([128, H], F32)
            for eb in range(2):
                pb2 = ps_pj.tile([128, 512], F32, tag="pj", name="pb2")
                nc.tensor.matmul(pb2[:], ones[0:1, :],
                                 bo4r[:, ts(eb, 512)])
                nc.vector.tensor_copy(bob[:, ts(eb, 512)], pb2[:])

            # ---- attention + output projection, per sq block.
            # outproj(sqb) is emitted after (hp0, sqb+1)'s attention so the
            # PE prioritizes feeding ACT at sq-block boundaries; the last
            # block's outproj is fused chunk-wise into its finish to cut
            # the drain tail. ----
            # ---- flat attention stream with score lookahead: inline
            # score+exp emission runs 2 items ahead of AV consumption, ALSO
            # across batch boundaries, so ACT keeps chewing exps while the
            # next batch's AVs wait out the previous normalize's PSUM reads
            # (pav is only double-buffered within a batch). outproj(sqb-1)
            # is deferred 3 items into the following batch for the same
            # reason. ----
            attn_finish(0, 0, pav00)
            batches = [(s, h) for s in range(NSB) for h in range(2)][1:]
            stream = [(s, h, k) for (s, h) in batches for k in range(SKT)]
            emitted = {}
            st_la = {"ep": 0}

            def pump(cp):
                while st_la["ep"] < len(stream) and st_la["ep"] <= cp + 2:
                    key = stream[st_la["ep"]]
                    if key not in pre:
                        emitted[key] = qk_exp(key[1], key[0], key[2])
                    st_la["ep"] += 1

            deferred = []
            pav_cur = {}
            for ci, key in enumerate(stream):
                sqb, hp, sk = key
                if sk == 0:
                    pav_cur[(sqb, hp)] = [
                        ps_av.tile([128, 4, 128], F32, tag="av",
                                   name=f"pav{hh}") for hh in range(2)]
                pump(ci)
                et = pre.pop(key, None)
                if et is None:
                    et = emitted.pop(key)
                av_accum(hp, sk, et, pav_cur[(sqb, hp)])
                if sk == 2 and deferred:
                    emit_outproj(deferred.pop())
                if sk == SKT - 1:
                    last = (sqb == NSB - 1 and hp == 1)
                    attn_finish(hp, sqb, pav_cur.pop((sqb, hp)),
                                fuse_outproj=last)
                    if hp == 0 and sqb > 0:
                        deferred.append(sqb - 1)

    nc.compile()
    return nc


def _get_nc():
    if "nc" not in _CACHE:
        _CACHE["nc"] = _build()
    return _CACHE["nc"]


def _kernel_numpy(query, key, value, attention_mask,
                  Wq, bq, Wk, bk, Wv, bv, Wo, bo):
    """Exact fp32 numpy fallback (only used for inputs outside the spec:
    nonzero mask or unexpected shapes)."""
    B, S_, H_ = query.shape
    NH = 16
    HDl = H_ // NH
    q = query @ Wq + bq
    k = key @ Wk + bk
    v = value @ Wv + bv

    def split(x):
        return x.reshape(B, S_, NH, HDl).transpose(0, 2, 1, 3)

    q, k, v = split(q), split(k), split(v)
    s = np.einsum("bhqd,bhkd->bhqk", q, k) / np.sqrt(np.float32(HDl))
    s = s + attention_mask[:, None, :, :]
    s = s - s.max(axis=-1, keepdims=True)
    e = np.exp(s)
    w = e / e.sum(axis=-1, keepdims=True)
    o = np.einsum("bhqk,bhkd->bhqd", w, v)
    o = o.transpose(0, 2, 1, 3).reshape(B, S_, H_)
    return (o @ Wo + bo).astype(np.float32)


def kernel(query, key, value, attention_mask, Wq, bq, Wk, bk, Wv, bv, Wo, bo):
    query = np.asarray(query, np.float32)
    key = np.asarray(key, np.float32)
    value = np.asarray(value, np.float32)
    Wq, Wk, Wv, Wo = (np.asarray(a, np.float32) for a in (Wq, Wk, Wv, Wo))
    bq, bk, bv, bo = (np.asarray(a, np.float32) for a in (bq, bk, bv, bo))
    attention_mask = np.asarray(attention_mask, np.float32)

    if query.shape != (2, S, H) or Wq.shape != (H, H) or \
            attention_mask.shape != (2, S, S) or np.any(attention_mask):
        return _kernel_numpy(query, key, value, attention_mask,
                             Wq, bq, Wk, bk, Wv, bv, Wo, bo)

    qT = [np.ascontiguousarray(query[b].astype(ml_dtypes.bfloat16).T)
          for b in range(2)]
    kTh = [np.ascontiguousarray(key[b].astype(ml_dtypes.bfloat16).T)
           for b in range(2)]
    vTh = [np.ascontiguousarray(value[b].astype(ml_dtypes.bfloat16).T)
           for b in range(2)]

    nc = _get_nc()
    in_maps = []
    for c in range(N_CORES):
        b, hg = divmod(c, 4)
        sl = slice(D * hg, D * hg + D)
        in_maps.append({
            "xqT": qT[b],
            "xkT": kTh[b],
            "xvT": vTh[b],
            "wq": np.ascontiguousarray(Wq[:, sl]).astype(ml_dtypes.bfloat16),
            "wk": np.ascontiguousarray(Wk[:, sl]).astype(ml_dtypes.bfloat16),
            "wv": np.ascontiguousarray(Wv[:, sl]).astype(ml_dtypes.bfloat16),
            "wo": np.ascontiguousarray(Wo[sl, :]).astype(ml_dtypes.bfloat16),
            "bq2": bq[sl].reshape(2, 128).T.copy(),
            "bk2": bk[sl].reshape(2, 128).T.copy(),
            "bv1": bv[sl].reshape(1, D).copy(),
            "bo4": (bo / 4.0).reshape(1, H),
        })
    try:
        res = run_bass_kernel_spmd(nc, in_maps, list(range(N_CORES)))
    finally:
        # run_bass_via_pjrt monkeypatches libneuronxla.neuronx_cc; restore it
        # so later ordinary jax compiles in the caller's process are untouched.
        try:
            import libneuronxla  # pyright: ignore[reportMissingImports]
            if hasattr(libneuronxla, "orig_neuronx_cc"):
                libneuronxla.neuronx_cc = libneuronxla.orig_neuronx_cc
        except ImportError:
            pass
    outs = [res.results[c]["y"] for c in range(N_CORES)]
    out = np.empty((2, S, H), np.float32)
    for b in range(2):
        out[b] = outs[4 * b] + outs[4 * b + 1] + outs[4 * b + 2] + outs[4 * b + 3]
    return out

